# revision 9
# baseline (speedup 1.0000x reference)
"""Trainium2 Bass kernel for nn_BasicClassifier (spiking conv classifier).

Sharding: pure data parallelism — batch 256 is split 32 samples per core
across 8 NeuronCores; params are replicated (tiny).

Per-core design. The T=1000 LIF scan is sequential, so per-step instruction
cost dominates. State lives in one [128, 128] fp32 SBUF tile:
  cols 0:96   layer-1 membrane, feature f = g*128+p at (p, g*32+b), g in 0..2
  cols 96:128 layer-2 membrane [35 units x 32 samples], lagged SKEW ticks
Per tick the whole LIF update (leak + input + reset-by-subtraction) is ONE
fused custom-DVE op:  m' = (m*0.9 + c) - (m > 1)   (the spike derives from
state, so the DVE chain never waits on other engines).

Feeding it, per 16-tick block:
  - conv1d hoisted into [31 -> 384] fp32r GEMMs (ones row folds conv_b):
    3 matmuls of N=512 into contiguous PSUM banks (full fp32r rate).
  - per tick: layer-1 spikes (m > 1) on the Pool engine; fc 384->35 as
    3 fp32r matmuls accumulating onto a bias-prefilled PSUM bank.
  - the ACT engine assembles the per-tick interleaved C tile in SBUF from
    the conv banks + the fc bank of 2 blocks ago (SKEW=32).
  - sum_t mem2 accumulates via an identity matmul into a PSUM bank.
"""

import os
import sys

for _p in ("/opt/trn_rl_repo", "/opt/pypackages"):
    if _p not in sys.path:
        sys.path.insert(0, _p)

import numpy as np

import concourse.bacc as bacc
import concourse.mybir as mybir
import concourse.tile as tile
import concourse.dve_ops as dve_ops
from concourse.dve_spec import Spec, Src0, Src1, C0, C1, lower
from concourse.dve_uop import DveOpSpec
from concourse.bass_utils import run_bass_kernel_spmd

F32 = mybir.dt.float32
F32R = mybir.dt.float32r
ALU = mybir.AluOpType
AF = mybir.ActivationFunctionType

N_CORES = 8
B_FULL, T_FULL, L_IN = 256, 1000, 30
BC = B_FULL // N_CORES      # 32 samples per core
CH, LO = 16, 24
F = CH * LO                 # 384 features
G = 3                       # feature groups of 128
J = 35                      # fc outputs
KX = L_IN + 1               # conv contraction rows (30 taps + ones row)
BLK = 16                    # ticks per block (conv N = 16*32 = 512 = 1 bank)
SKEW = 2 * BLK              # layer-2 lag: c2_t consumed at DVE tick t+SKEW
WIN = 160                   # ticks per x-window DMA (multiple of BLK)
BETA, THR = 0.9, 1.0

TRACE = bool(int(os.environ.get("KERNEL_TRACE", "0")))
LAST_RESULTS = None

_LIF_OP = None


def _get_lif_op():
    """Register the fused LIF-step op in the custom-DVE table (idempotent)."""
    global _LIF_OP
    if _LIF_OP is not None:
        return _LIF_OP
    name = "LIF_STEP_ANT59"
    for op in dve_ops.OPS:
        if op.name == name:
            _LIF_OP = op
            return op
    spec = Spec(
        body=(Src0 * C0 + Src1) - (Src0 > C1),
        reference=lambda in0, in1, s0, s1, imm2: (
            (in0.astype(np.float32) * np.float32(s0) + in1)
            - (in0 > s1).astype(np.float32)
        ).astype(np.float32),
    )
    row = dve_ops._CUSTOM_DVE_ROW_BASE + len(dve_ops.OPS)
    assert row < 0x20
    dve_ops._SUB_OPCODE_FOR_NAME[name] = row
    shas = {}
    for ver in ("v3", "v4"):
        tmp = DveOpSpec(name=name, opcode=row, uops=lower(spec, ver=ver), rd1_en=True)
        shas[ver] = tmp.sha(ver)
    op = dve_ops.DveOp(name, spec, subdim=False, uops_sha=shas)
    dve_ops.OPS.append(op)
    dve_ops.CUSTOM_DVE_SPECS[name] = spec
    _LIF_OP = op
    return op


def _build_nc(T):
    """Build the per-core Bass program (SPMD: same program on every core)."""
    lif = _get_lif_op()
    ticks = T + SKEW                       # DVE ticks 0..T+SKEW-1
    nblk = -(-ticks // BLK)
    pad_ticks = nblk * BLK
    windows = -(-pad_ticks // WIN)
    xt_cols = windows * WIN * BC

    nc = bacc.Bacc("TRN2", target_bir_lowering=False)

    xt_d = nc.dram_tensor("xt", [KX, xt_cols], F32, kind="ExternalInput")
    wexp_d = nc.dram_tensor("wexp", [KX, F], F32, kind="ExternalInput")
    fcwt_d = nc.dram_tensor("fcwt", [128, G * J], F32, kind="ExternalInput")
    brow_d = nc.dram_tensor("brow", [1, 128], F32, kind="ExternalInput")
    ones_d = nc.dram_tensor("ones", [1, BLK * BC], F32, kind="ExternalInput")
    eye_d = nc.dram_tensor("eye", [J, J], F32, kind="ExternalInput")
    out_d = nc.dram_tensor("out", [J, BC], F32, kind="ExternalOutput")

    with tile.TileContext(nc) as tc:
        with (
            tc.tile_pool(name="konst", bufs=1) as kp,
            tc.tile_pool(name="state", bufs=1) as sp,
            tc.tile_pool(name="xwin", bufs=2) as xp,
            tc.tile_pool(name="cchunk", bufs=2) as cp,
            tc.tile_pool(name="psumv", bufs=3, space="PSUM") as pv,
            tc.tile_pool(name="psumf", bufs=3, space="PSUM") as pf,
            tc.tile_pool(name="psuma", bufs=1, space="PSUM") as pa,
        ):
            # constants -> SBUF
            wexp = kp.tile([KX, F], F32, tag="wexp")
            fcwt = kp.tile([128, G * J], F32, tag="fcwt")
            brow = kp.tile([1, 128], F32, tag="brow")
            ones = kp.tile([1, BLK * BC], F32, tag="ones")
            eye = kp.tile([J, J], F32, tag="eye")
            nc.sync.dma_start(wexp[:], wexp_d[:])
            nc.sync.dma_start(fcwt[:], fcwt_d[:])
            nc.sync.dma_start(brow[:], brow_d[:])
            nc.sync.dma_start(ones[:], ones_d[:])
            nc.sync.dma_start(eye[:], eye_d[:])

            # persistent state (double-buffered) + spike tiles
            stA = sp.tile([128, 128], F32, tag="stA")
            stB = sp.tile([128, 128], F32, tag="stB")
            sgA = sp.tile([128, G * BC], F32, tag="sgA")
            sgB = sp.tile([128, G * BC], F32, tag="sgB")
            nc.vector.memset(stA[:], 0.0)
            nc.vector.memset(stB[:], 0.0)

            acc = pa.tile([J, BC], F32, tag="acc")

            xts = {}      # window idx -> xt sbuf tile
            fcs = {}      # block idx -> fc psum tile [128, 512]
            chs = {}      # block idx -> assembled sbuf C tile [128, BLK*128]

            def load_window(w):
                if w >= windows or w in xts:
                    return
                t0 = xp.tile([KX, WIN * BC], F32, tag="xw")
                nc.sync.dma_start(t0[:], xt_d[:, w * WIN * BC:(w + 1) * WIN * BC])
                xts[w] = t0

            def prep_block(b):
                """Fill conv banks for block b, assemble its SBUF C tile, and
                allocate + bias-prefill its fc PSUM bank."""
                if b >= nblk or b in chs:
                    return
                w = (b * BLK) // WIN
                base = (b * BLK - w * WIN) * BC
                ch = cp.tile([128, BLK * 128], F32, tag="ch")
                chs[b] = ch
                ch3 = ch[:].rearrange("p (t c) -> p t c", c=128)
                for g in range(G):
                    pvg = pv.tile([128, BLK * BC], F32, tag="pv")
                    nc.tensor.matmul(
                        out=pvg[:],
                        lhsT=wexp[:, g * 128:(g + 1) * 128],
                        rhs=xts[w][:, base:base + BLK * BC],
                        start=True, stop=True,
                    )
                    # scatter conv bank into per-tick layout: cols g*32..g*32+32
                    nc.scalar.activation(
                        out=ch3[:, :, g * BC:(g + 1) * BC], in_=pvg[:], func=AF.Copy,
                    )
                # c2 slots: block b consumes c2 of block b-2
                if b >= 2:
                    nc.scalar.activation(
                        out=ch3[:, :, G * BC:128], in_=fcs[b - 2][:], func=AF.Copy,
                    )
                else:
                    nc.vector.memset(ch3[:, :, G * BC:128], 0.0)
                # fc accumulator bank for block b: prefill with fc bias
                fcb = pf.tile([128, BLK * BC], F32, tag="pf")
                fcs[b] = fcb
                nc.tensor.matmul(
                    out=fcb[:], lhsT=brow[0:1, :], rhs=ones[0:1, :],
                    start=True, stop=True,
                )

            # prologue
            load_window(0)
            prep_block(0)
            prep_block(1)

            for t in range(ticks):
                b, lo = divmod(t, BLK)
                if lo == 0:
                    if ((b + 3) * BLK) % WIN == 0:
                        load_window(((b + 3) * BLK) // WIN)
                    prep_block(b + 1)
                cur, prv = (stA, stB) if t % 2 == 0 else (stB, stA)
                sg = sgA if t % 2 == 0 else sgB

                # fused LIF step for both layers
                nc.vector._custom_dve(
                    lif,
                    out=cur[:],
                    in0=prv[:],
                    in1=chs[b][:, lo * 128:(lo + 1) * 128],
                    s0=BETA, s1=THR,
                )

                if t < T:
                    # layer-1 spikes on Pool
                    nc.gpsimd.tensor_scalar(
                        out=sg[:], in0=cur[:, 0:G * BC],
                        scalar1=THR, scalar2=None, op0=ALU.is_gt,
                    )
                    # fc: c2_t = fc bias + W @ spikes, into this block's fc bank
                    for g in range(G):
                        nc.tensor.matmul(
                            out=fcs[b][0:J, lo * BC:(lo + 1) * BC],
                            lhsT=fcwt[:, g * J:(g + 1) * J],
                            rhs=sg[:, g * BC:(g + 1) * BC],
                            start=False, stop=(g == G - 1),
                            skip_group_check=True,
                        )
                if SKEW <= t:
                    # acc += mem2_{t-SKEW}
                    nc.tensor.matmul(
                        out=acc[:],
                        lhsT=eye[:],
                        rhs=cur[0:J, G * BC:128],
                        start=(t == SKEW), stop=(t == ticks - 1),
                        skip_group_check=True,
                    )

            acc_sb = kp.tile([J, BC], F32, tag="acc_sb")
            nc.vector.tensor_copy(acc_sb[:], acc[:])
            nc.sync.dma_start(out_d[:], acc_sb[:])

    nc.compile()
    return nc


def _host_prep(x, conv_w, conv_b, fc_w, fc_b, T):
    """Build per-core input maps (numpy only)."""
    ticks = T + SKEW
    nblk = -(-ticks // BLK)
    windows = -(-(nblk * BLK) // WIN)
    xt_ticks = windows * WIN

    wexp = np.zeros((KX, F), np.float32)
    for c in range(CH):
        for l in range(LO):
            wexp[l:l + 7, c * LO + l] = conv_w[c, 0, :]
        wexp[L_IN, c * LO:(c + 1) * LO] = conv_b[c]

    fcwt = np.zeros((128, G * J), np.float32)
    for g in range(G):
        fcwt[:, g * J:(g + 1) * J] = fc_w[:, g * 128:(g + 1) * 128].T

    brow = np.zeros((1, 128), np.float32)
    brow[0, :J] = fc_b
    ones = np.ones((1, BLK * BC), np.float32)
    eye = np.eye(J, dtype=np.float32)

    in_maps = []
    B = x.shape[0]
    n_cores = B // BC
    for core in range(n_cores):
        xc = x[core * BC:(core + 1) * BC]          # [BC, T, L]
        xt = np.zeros((KX, xt_ticks, BC), np.float32)
        xt[:L_IN, :T, :] = xc.transpose(2, 1, 0)
        xt[L_IN, :T, :] = 1.0
        in_maps.append({
            "xt": np.ascontiguousarray(xt.reshape(KX, xt_ticks * BC)),
            "wexp": wexp, "fcwt": fcwt, "brow": brow,
            "ones": ones, "eye": eye,
        })
    return in_maps


def _install_trace_hook():
    """Wire up the axon NTFF profiling hook (absent from this image)."""
    import types

    if "antenv.axon_hooks" in sys.modules:
        return True
    try:
        if "/root/.axon_site" not in sys.path:
            sys.path.insert(0, "/root/.axon_site")
        from trn_agent_boot.trn_boot import _ntff_profile_via_ctypes

        hook = _ntff_profile_via_ctypes("/opt/axon/libaxon_pjrt.so")
        if hook is None:
            return False
        mod = types.ModuleType("antenv.axon_hooks")
        mod.get_axon_ntff_profile_hook = lambda: hook
        sys.modules["antenv.axon_hooks"] = mod
        import concourse.bass_utils as bu

        bu.upload_artifacts = lambda tmpdir: str(tmpdir)
        return True
    except Exception as e:  # profiling is optional
        print(f"trace hook install failed: {e}", file=sys.stderr)
        return False


def run_cores(x, conv_w, conv_b, fc_w, fc_b, T=None):
    """Run the Bass kernel on len(batch)/32 cores; returns [B, 35] output."""
    global LAST_RESULTS
    T = T if T is not None else x.shape[1]
    trace = TRACE and _install_trace_hook()
    nc = _build_nc(T)
    in_maps = _host_prep(x, conv_w, conv_b, fc_w, fc_b, T)
    res = run_bass_kernel_spmd(
        nc, in_maps, core_ids=list(range(len(in_maps))), trace=trace,
    )
    LAST_RESULTS = res
    outs = []
    for i in range(len(in_maps)):
        acc_v = res.results[i]["out"]              # [J, BC] = sum_t mem2
        outs.append((acc_v.T / np.float32(T)).astype(np.float32))
    return np.concatenate(outs, axis=0)


def kernel(x, conv_w, conv_b, fc_w, fc_b):
    return run_cores(
        np.asarray(x, np.float32), np.asarray(conv_w, np.float32),
        np.asarray(conv_b, np.float32), np.asarray(fc_w, np.float32),
        np.asarray(fc_b, np.float32),
    )


# revision 14
# speedup vs baseline: 2.8378x; 2.8378x over previous
"""Trainium2 Bass kernel for nn_BasicClassifier (spiking conv classifier).

Sharding: pure data parallelism — batch 256 is split 32 samples per core
across 8 NeuronCores; params are replicated (tiny).

Per-core design. The T=1000 LIF scan is sequential, so per-step cost on the
pacing engine decides everything. State lives in a ring of [128, 128] fp32
SBUF slices (2 block-sized ring tiles, 16 slices each):
  cols 0:96   layer-1 membrane, feature f = g*128+p at (p, g*32+b), g in 0..2
  cols 96:128 layer-2 membrane [35 units x 32 samples], lagged SKEW=32 ticks
Per tick the whole LIF update (leak + input + reset-by-subtraction) is ONE
fused custom-DVE op:  m' = (m*0.9 + c) - (m > 1)   (the spike derives from
state, so the DVE chain never waits on other engines mid-block).

Per 16-tick block (everything else is block-batched off the critical path):
  - conv1d hoisted into [31 -> 384] fp32 GEMMs (ones row folds conv_b):
    3 matmuls of N=512 into PSUM banks; ACT scatters them into the
    per-tick-interleaved SBUF C tile.
  - spikes: ONE ACT Sign op per block over the ring -> sigma in {-1,0,1};
    (sigma+1)/2 is folded into halved fc weights + adjusted fc bias.
  - fc 384->35: 3 fp32 matmuls of N=512 accumulating onto a bias-prefilled
    PSUM bank; ACT scatters the bank of block b into C tile of block b+2.
  - mem2 history: ACT copies ring cols 96:128 to an fp16 [35, 32*T] buffer;
    one final DVE tensor_reduce produces sum_t mem2.
"""

import os
import sys

for _p in ("/opt/trn_rl_repo", "/opt/pypackages"):
    if _p not in sys.path:
        sys.path.insert(0, _p)

import numpy as np

import concourse.bacc as bacc
import concourse.mybir as mybir
import concourse.tile as tile
import concourse.dve_ops as dve_ops
from concourse.dve_spec import Spec, Src0, Src1, C0, C1, lower
from concourse.dve_uop import DveOpSpec
from concourse.bass_utils import run_bass_kernel_spmd

F32 = mybir.dt.float32
F16 = mybir.dt.float16
ALU = mybir.AluOpType
AF = mybir.ActivationFunctionType
AX = mybir.AxisListType

N_CORES = 8
B_FULL, T_FULL, L_IN = 256, 1000, 30
BC = B_FULL // N_CORES      # 32 samples per core
CH, LO = 16, 24
F = CH * LO                 # 384 features
G = 3                       # feature groups of 128
J = 35                      # fc outputs
KX = L_IN + 1               # conv contraction rows (30 taps + ones row)
BLK = 16                    # ticks per block (N = 16*32 = 512 = 1 PSUM bank)
SKEW = 2 * BLK              # layer-2 lag: c2_t consumed at DVE tick t+SKEW
WIN = 160                   # ticks per x-window DMA (multiple of BLK)
BETA, THR = 0.9, 1.0

TRACE = bool(int(os.environ.get("KERNEL_TRACE", "0")))
LAST_RESULTS = None

_LIF_OP = None


def _get_lif_op():
    """Register the fused LIF-step op in the custom-DVE table (idempotent)."""
    global _LIF_OP
    if _LIF_OP is not None:
        return _LIF_OP
    name = "LIF_STEP_ANT59"
    for op in dve_ops.OPS:
        if op.name == name:
            _LIF_OP = op
            return op
    spec = Spec(
        body=(Src0 * C0 + Src1) - (Src0 > C1),
        reference=lambda in0, in1, s0, s1, imm2: (
            (in0.astype(np.float32) * np.float32(s0) + in1)
            - (in0 > s1).astype(np.float32)
        ).astype(np.float32),
    )
    row = dve_ops._CUSTOM_DVE_ROW_BASE + len(dve_ops.OPS)
    assert row < 0x20
    dve_ops._SUB_OPCODE_FOR_NAME[name] = row
    shas = {}
    for ver in ("v3", "v4"):
        tmp = DveOpSpec(name=name, opcode=row, uops=lower(spec, ver=ver), rd1_en=True)
        shas[ver] = tmp.sha(ver)
    op = dve_ops.DveOp(name, spec, subdim=False, uops_sha=shas)
    dve_ops.OPS.append(op)
    dve_ops.CUSTOM_DVE_SPECS[name] = spec
    _LIF_OP = op
    return op


def _build_nc(T):
    """Build the per-core Bass program (SPMD: same program on every core)."""
    lif = _get_lif_op()
    ticks = T + SKEW                       # DVE ticks 0..T+SKEW-1
    nblk = -(-ticks // BLK)
    pad_ticks = nblk * BLK
    windows = -(-pad_ticks // WIN)
    xt_cols = windows * WIN * BC
    fcblk = -(-T // BLK)                   # blocks that need spikes/fc

    nc = bacc.Bacc("TRN2", target_bir_lowering=False)

    xt_d = nc.dram_tensor("xt", [KX, xt_cols], F32, kind="ExternalInput")
    wexp_d = nc.dram_tensor("wexp", [KX, F], F32, kind="ExternalInput")
    fcwt_d = nc.dram_tensor("fcwt", [128, G * J], F32, kind="ExternalInput")
    brow_d = nc.dram_tensor("brow", [1, 128], F32, kind="ExternalInput")
    ones_d = nc.dram_tensor("ones", [1, BLK * BC], F32, kind="ExternalInput")
    out_d = nc.dram_tensor("out", [J, BC], F32, kind="ExternalOutput")

    with tile.TileContext(nc) as tc:
        with (
            tc.tile_pool(name="konst", bufs=1) as kp,
            tc.tile_pool(name="ring", bufs=1) as rp,
            tc.tile_pool(name="hist", bufs=1) as hp,
            tc.tile_pool(name="sig", bufs=2) as sgp,
            tc.tile_pool(name="xwin", bufs=2) as xp,
            tc.tile_pool(name="cchunk", bufs=2) as cp,
            tc.tile_pool(name="psumv", bufs=3, space="PSUM") as pv,
            tc.tile_pool(name="psumf", bufs=3, space="PSUM") as pf,
        ):
            # constants -> SBUF
            wexp = kp.tile([KX, F], F32, tag="wexp")
            fcwt = kp.tile([128, G * J], F32, tag="fcwt")
            brow = kp.tile([1, 128], F32, tag="brow")
            ones = kp.tile([1, BLK * BC], F32, tag="ones")
            negthr = kp.tile([128, 1], F32, tag="negthr")
            nc.vector.memset(negthr[:], -THR)
            nc.sync.dma_start(wexp[:], wexp_d[:])
            nc.sync.dma_start(fcwt[:], fcwt_d[:])
            nc.sync.dma_start(brow[:], brow_d[:])
            nc.sync.dma_start(ones[:], ones_d[:])

            # state ring: 2 block-sized tiles of 16 slices each
            ringA = rp.tile([128, BLK * 128], F32, tag="ringA")
            ringB = rp.tile([128, BLK * 128], F32, tag="ringB")
            nc.vector.memset(ringA[:], 0.0)
            nc.vector.memset(ringB[:], 0.0)
            rings = (ringA, ringB)

            # mem2 history (bf16), col = sample*T + t
            hist = hp.tile([J, BC * T], F16, tag="hist")

            xts = {}      # window idx -> xt sbuf tile
            fcs = {}      # block idx -> fc psum tile [128, 512]
            chs = {}      # block idx -> assembled sbuf C tile [128, BLK*128]

            def load_window(w):
                if w >= windows or w in xts:
                    return
                t0 = xp.tile([KX, WIN * BC], F32, tag="xw")
                nc.sync.dma_start(t0[:], xt_d[:, w * WIN * BC:(w + 1) * WIN * BC])
                xts[w] = t0

            def prep_block(b):
                """Conv-fill + assemble the SBUF C tile of block b; allocate
                and bias-prefill its fc PSUM bank (c2 source is block b-2)."""
                if b >= nblk or b in chs:
                    return
                w = (b * BLK) // WIN
                base = (b * BLK - w * WIN) * BC
                ch = cp.tile([128, BLK * 128], F32, tag="ch")
                chs[b] = ch
                ch3 = ch[:].rearrange("p (t c) -> p t c", c=128)
                for g in range(G):
                    pvg = pv.tile([128, BLK * BC], F32, tag="pv")
                    nc.tensor.matmul(
                        out=pvg[:],
                        lhsT=wexp[:, g * 128:(g + 1) * 128],
                        rhs=xts[w][:, base:base + BLK * BC],
                        start=True, stop=True,
                    )
                    nc.scalar.activation(
                        out=ch3[:, :, g * BC:(g + 1) * BC], in_=pvg[:], func=AF.Copy,
                    )
                if b >= 2:
                    nc.scalar.activation(
                        out=ch3[:, :, G * BC:128], in_=fcs[b - 2][:], func=AF.Copy,
                    )
                else:
                    nc.vector.memset(ch3[:, :, G * BC:128], 0.0)
                if b < fcblk:
                    fcb = pf.tile([128, BLK * BC], F32, tag="pf")
                    fcs[b] = fcb
                    nc.tensor.matmul(
                        out=fcb[:], lhsT=brow[0:1, :], rhs=ones[0:1, :],
                        start=True, stop=True,
                    )

            def spikes_and_fc(b):
                """After block b's ticks: sigma = Sign(m1 - 1) for its 16
                slices, then fc for its 16 c2 columns (3 matmuls, N=512)."""
                if b < 0 or b >= fcblk:
                    return
                ring3 = rings[b % 2][:].rearrange("p (t c) -> p t c", c=128)
                sg = sgp.tile([128, BLK * G * BC], F32, tag="sg")
                sg3 = sg[:].rearrange("p (t c) -> p t c", c=G * BC)
                nc.scalar.activation(
                    out=sg3[:], in_=ring3[:, :, 0:G * BC], func=AF.Sign, bias=negthr[:],
                )
                for g in range(G):
                    nc.tensor.matmul(
                        out=fcs[b][0:J, :],
                        lhsT=fcwt[:, g * J:(g + 1) * J],
                        rhs=sg3[:, :, g * BC:(g + 1) * BC],
                        start=False, stop=(g == G - 1),
                        skip_group_check=True,
                    )

            def hist_copy(b):
                """mem2 of DVE-tick block b = m2 ticks [16b-32, 16b-16)."""
                t0 = b * BLK - SKEW
                if t0 < 0:
                    return
                n = min(BLK, T - t0)
                if n <= 0:
                    return
                ring3 = rings[b % 2][:].rearrange("p (t c) -> p t c", c=128)
                hview = hist[:].rearrange("p (s t) -> p s t", t=T)
                # out[j, sample, t0:t0+n] <- ring slice u cols 96:128
                nc.scalar.activation(
                    out=hview[0:J, :, t0:t0 + n].transpose((0, 2, 1)),
                    in_=ring3[0:J, 0:n, G * BC:128], func=AF.Copy,
                )

            # prologue
            load_window(0)
            prep_block(0)
            prep_block(1)

            for t in range(ticks):
                b, lo = divmod(t, BLK)
                if lo == 0:
                    if ((b + 3) * BLK) % WIN == 0:
                        load_window(((b + 3) * BLK) // WIN)
                    spikes_and_fc(b - 1)
                    hist_copy(b - 1)
                    prep_block(b + 1)
                ring = rings[b % 2]
                prev = rings[(b - 1) % 2] if lo == 0 else ring
                plo = (lo - 1) % BLK

                nc.vector._custom_dve(
                    lif,
                    out=ring[:, lo * 128:(lo + 1) * 128],
                    in0=prev[:, plo * 128:(plo + 1) * 128],
                    in1=chs[b][:, lo * 128:(lo + 1) * 128],
                    s0=BETA, s1=THR,
                )

            # epilogue: last two blocks' spikes are never needed, but their
            # mem2 history is
            spikes_and_fc(nblk - 1)
            hist_copy(nblk - 1)
            # (hist for the final partial block, if ticks pad past T+SKEW)

            acc_sb = kp.tile([J, BC], F32, tag="acc_sb")
            hview = hist[:].rearrange("p (s t) -> p s t", t=T)
            nc.vector.tensor_reduce(
                out=acc_sb[:], in_=hview[:], axis=AX.X, op=ALU.add,
            )
            nc.sync.dma_start(out_d[:], acc_sb[:])

    nc.compile()
    return nc


def _host_prep(x, conv_w, conv_b, fc_w, fc_b, T):
    """Build per-core input maps (numpy only)."""
    ticks = T + SKEW
    nblk = -(-ticks // BLK)
    windows = -(-(nblk * BLK) // WIN)
    xt_ticks = windows * WIN

    wexp = np.zeros((KX, F), np.float32)
    for c in range(CH):
        for l in range(LO):
            wexp[l:l + 7, c * LO + l] = conv_w[c, 0, :]
        wexp[L_IN, c * LO:(c + 1) * LO] = conv_b[c]

    # spike trick: s = (sigma+1)/2 with sigma = sign(m-1) in {-1,0,1}
    # c2 = fc_w @ s + b = (fc_w/2) @ sigma + (b + fc_w.sum/2)
    fcwt = np.zeros((128, G * J), np.float32)
    half = (fc_w * 0.5).astype(np.float32)
    for g in range(G):
        fcwt[:, g * J:(g + 1) * J] = half[:, g * 128:(g + 1) * 128].T
    brow = np.zeros((1, 128), np.float32)
    brow[0, :J] = fc_b + half.sum(axis=1)

    ones = np.ones((1, BLK * BC), np.float32)

    in_maps = []
    B = x.shape[0]
    n_cores = B // BC
    for core in range(n_cores):
        xc = x[core * BC:(core + 1) * BC]          # [BC, T, L]
        xt = np.zeros((KX, xt_ticks, BC), np.float32)
        xt[:L_IN, :T, :] = xc.transpose(2, 1, 0)
        xt[L_IN, :T, :] = 1.0
        in_maps.append({
            "xt": np.ascontiguousarray(xt.reshape(KX, xt_ticks * BC)),
            "wexp": wexp, "fcwt": fcwt, "brow": brow, "ones": ones,
        })
    return in_maps


def _install_trace_hook():
    """Wire up the axon NTFF profiling hook (absent from this image)."""
    import types

    if "antenv.axon_hooks" in sys.modules:
        return True
    try:
        if "/root/.axon_site" not in sys.path:
            sys.path.insert(0, "/root/.axon_site")
        from trn_agent_boot.trn_boot import _ntff_profile_via_ctypes

        hook = _ntff_profile_via_ctypes("/opt/axon/libaxon_pjrt.so")
        if hook is None:
            return False
        mod = types.ModuleType("antenv.axon_hooks")
        mod.get_axon_ntff_profile_hook = lambda: hook
        sys.modules["antenv.axon_hooks"] = mod
        import concourse.bass_utils as bu

        bu.upload_artifacts = lambda tmpdir: str(tmpdir)
        return True
    except Exception as e:  # profiling is optional
        print(f"trace hook install failed: {e}", file=sys.stderr)
        return False


def run_cores(x, conv_w, conv_b, fc_w, fc_b, T=None):
    """Run the Bass kernel on len(batch)/32 cores; returns [B, 35] output."""
    global LAST_RESULTS
    T = T if T is not None else x.shape[1]
    trace = TRACE and _install_trace_hook()
    nc = _build_nc(T)
    in_maps = _host_prep(x, conv_w, conv_b, fc_w, fc_b, T)
    res = run_bass_kernel_spmd(
        nc, in_maps, core_ids=list(range(len(in_maps))), trace=trace,
    )
    LAST_RESULTS = res
    outs = []
    for i in range(len(in_maps)):
        acc_v = res.results[i]["out"]              # [J, BC] = sum_t mem2
        outs.append((acc_v.T / np.float32(T)).astype(np.float32))
    return np.concatenate(outs, axis=0)


def kernel(x, conv_w, conv_b, fc_w, fc_b):
    return run_cores(
        np.asarray(x, np.float32), np.asarray(conv_w, np.float32),
        np.asarray(conv_b, np.float32), np.asarray(fc_w, np.float32),
        np.asarray(fc_b, np.float32),
    )


# revision 17
# speedup vs baseline: 3.3223x; 1.1708x over previous
"""Trainium2 Bass kernel for nn_BasicClassifier (spiking conv classifier).

Sharding: pure data parallelism — batch 256 is split 32 samples per core
across 8 NeuronCores; params are replicated (tiny).

Per-core design. The T=1000 LIF scan is sequential, so per-step cost on the
pacing engine decides everything. State lives in a ring of [128, 128] fp32
SBUF slices (2 block-sized ring tiles, 16 slices each):
  cols 0:96   layer-1 membrane, feature f = g*128+p at (p, g*32+b), g in 0..2
  cols 96:128 layer-2 membrane [35 units x 32 samples], lagged SKEW=32 ticks
Per tick the whole LIF update (leak + input + reset-by-subtraction) is ONE
fused custom-DVE op:  m' = (m*0.9 + c) - (m > 1)   (the spike derives from
state, so the DVE chain never waits on other engines mid-block). Its input
drive c is read straight out of a 4-bank PSUM block tile via a strided
[128, 4, 32] access pattern — no assembly copies.

Per 16-tick block (all off the tick-critical path, bf16 hi/lo splits keep
matmuls single-pass while preserving ~2^-16 relative precision):
  - C tile = PSUM [128, 4*512]: banks 0-2 = conv1d hoisted into [31 -> 384]
    GEMMs (3 products: xh@Wh + xl@Wh + xh@Wl; ones row folds conv_b),
    bank 3 = fc bias prefill (bh+bl) + fc results of block b-2.
  - spikes: ONE ACT Sign op per block over the ring -> sigma in {-1,0,1} bf16;
    (sigma+1)/2 is folded into halved fc weights + adjusted fc bias.
  - fc 384->35: 6 bf16 matmuls of N=512 (hi/lo x 3 K-chunks) accumulating
    onto the bias in bank 3 of the C tile two blocks ahead.
  - mem2 history: ACT copies ring cols 96:128 to an fp16 [35, 32*T] buffer;
    one final DVE tensor_reduce produces sum_t mem2.
"""

import os
import sys

for _p in ("/opt/trn_rl_repo", "/opt/pypackages"):
    if _p not in sys.path:
        sys.path.insert(0, _p)

import numpy as np

import concourse.bacc as bacc
import concourse.mybir as mybir
import concourse.tile as tile
import concourse.dve_ops as dve_ops
from concourse.dve_spec import Spec, Src0, Src1, C0, C1, lower
from concourse.dve_uop import DveOpSpec
from concourse.bass_utils import run_bass_kernel_spmd

F32 = mybir.dt.float32
F16 = mybir.dt.float16
BF16 = mybir.dt.bfloat16
ALU = mybir.AluOpType
AF = mybir.ActivationFunctionType
AX = mybir.AxisListType

N_CORES = 8
B_FULL, T_FULL, L_IN = 256, 1000, 30
BC = B_FULL // N_CORES      # 32 samples per core
CH, LO = 16, 24
F = CH * LO                 # 384 features
G = 3                       # feature groups of 128
J = 35                      # fc outputs
KX = L_IN + 1               # conv contraction rows (30 taps + ones row)
BLK = 16                    # ticks per block (N = 16*32 = 512 = 1 PSUM bank)
SKEW = 2 * BLK              # layer-2 lag: c2_t consumed at DVE tick t+SKEW
WIN = 160                   # ticks per x-window DMA (multiple of BLK)
BETA, THR = 0.9, 1.0

TRACE = bool(int(os.environ.get("KERNEL_TRACE", "0")))
LAST_RESULTS = None

_LIF_OP = None


def _get_lif_op():
    """Register the fused LIF-step op in the custom-DVE table (idempotent)."""
    global _LIF_OP
    if _LIF_OP is not None:
        return _LIF_OP
    name = "LIF_STEP_ANT59"
    for op in dve_ops.OPS:
        if op.name == name:
            _LIF_OP = op
            return op
    spec = Spec(
        body=(Src0 * C0 + Src1) - (Src0 > C1),
        reference=lambda in0, in1, s0, s1, imm2: (
            (in0.astype(np.float32) * np.float32(s0)
             + in1.reshape(in0.shape))
            - (in0 > s1).astype(np.float32)
        ).astype(np.float32),
    )
    row = dve_ops._CUSTOM_DVE_ROW_BASE + len(dve_ops.OPS)
    assert row < 0x20
    dve_ops._SUB_OPCODE_FOR_NAME[name] = row
    shas = {}
    for ver in ("v3", "v4"):
        tmp = DveOpSpec(name=name, opcode=row, uops=lower(spec, ver=ver), rd1_en=True)
        shas[ver] = tmp.sha(ver)
    op = dve_ops.DveOp(name, spec, subdim=False, uops_sha=shas)
    dve_ops.OPS.append(op)
    dve_ops.CUSTOM_DVE_SPECS[name] = spec
    _LIF_OP = op
    return op


def _build_nc(T):
    """Build the per-core Bass program (SPMD: same program on every core)."""
    lif = _get_lif_op()
    ticks = T + SKEW                       # DVE ticks 0..T+SKEW-1
    nblk = -(-ticks // BLK)
    pad_ticks = nblk * BLK
    windows = -(-pad_ticks // WIN)
    xt_cols = windows * WIN * BC
    fcblk = -(-T // BLK)                   # blocks that need spikes/fc
    NB = BLK * BC                          # 512: one PSUM bank of f32

    nc = bacc.Bacc("TRN2", target_bir_lowering=False)

    xth_d = nc.dram_tensor("xth", [KX, xt_cols], BF16, kind="ExternalInput")
    xtl_d = nc.dram_tensor("xtl", [KX, xt_cols], BF16, kind="ExternalInput")
    weh_d = nc.dram_tensor("weh", [KX, F], BF16, kind="ExternalInput")
    wel_d = nc.dram_tensor("wel", [KX, F], BF16, kind="ExternalInput")
    fch_d = nc.dram_tensor("fch", [128, G * J], BF16, kind="ExternalInput")
    fcl_d = nc.dram_tensor("fcl", [128, G * J], BF16, kind="ExternalInput")
    brh_d = nc.dram_tensor("brh", [1, 128], BF16, kind="ExternalInput")
    brl_d = nc.dram_tensor("brl", [1, 128], BF16, kind="ExternalInput")
    ones_d = nc.dram_tensor("ones", [1, NB], BF16, kind="ExternalInput")
    out_d = nc.dram_tensor("out", [J, BC], F32, kind="ExternalOutput")

    with tile.TileContext(nc) as tc:
        with (
            tc.tile_pool(name="konst", bufs=1) as kp,
            tc.tile_pool(name="ring", bufs=1) as rp,
            tc.tile_pool(name="hist", bufs=1) as hp,
            tc.tile_pool(name="sig", bufs=2) as sgp,
            tc.tile_pool(name="xwin", bufs=2) as xp,
            tc.tile_pool(name="cpsum", bufs=2, space="PSUM") as cp,
        ):
            # constants -> SBUF
            weh = kp.tile([KX, F], BF16, tag="weh")
            wel = kp.tile([KX, F], BF16, tag="wel")
            fch = kp.tile([128, G * J], BF16, tag="fch")
            fcl = kp.tile([128, G * J], BF16, tag="fcl")
            brh = kp.tile([1, 128], BF16, tag="brh")
            brl = kp.tile([1, 128], BF16, tag="brl")
            ones = kp.tile([1, NB], BF16, tag="ones")
            negthr = kp.tile([128, 1], F32, tag="negthr")
            nc.vector.memset(negthr[:], -THR)
            for sb, dr in ((weh, weh_d), (wel, wel_d), (fch, fch_d),
                           (fcl, fcl_d), (brh, brh_d), (brl, brl_d),
                           (ones, ones_d)):
                nc.sync.dma_start(sb[:], dr[:])

            # state ring: 2 block-sized tiles of 16 slices each
            ringA = rp.tile([128, BLK * 128], F32, tag="ringA")
            ringB = rp.tile([128, BLK * 128], F32, tag="ringB")
            nc.vector.memset(ringA[:], 0.0)
            nc.vector.memset(ringB[:], 0.0)
            rings = (ringA, ringB)

            # mem2 history (fp16), col = sample*T + t
            hist = hp.tile([J, BC * T], F16, tag="hist")

            xts = {}      # window idx -> (hi, lo) xt sbuf tiles
            chs = {}      # block idx -> PSUM C tile [128, 4*512]

            def load_window(w):
                if w >= windows or w in xts:
                    return
                th = xp.tile([KX, WIN * BC], BF16, tag="xwh")
                tl = xp.tile([KX, WIN * BC], BF16, tag="xwl")
                sl = slice(w * WIN * BC, (w + 1) * WIN * BC)
                nc.sync.dma_start(th[:], xth_d[:, sl])
                nc.sync.dma_start(tl[:], xtl_d[:, sl])
                xts[w] = (th, tl)

            def prep_block(b):
                """Allocate block b's PSUM C tile; fill conv banks 0-2 and
                bias-prefill bank 3 (fc of block b-2 lands there later)."""
                if b >= nblk or b in chs:
                    return
                w = (b * BLK) // WIN
                base = (b * BLK - w * WIN) * BC
                xh, xl = xts[w]
                ch = cp.tile([128, 4 * NB], F32, tag="ch")
                chs[b] = ch
                for g in range(G):
                    bank = ch[:, g * NB:(g + 1) * NB]
                    for i, (lw, rx) in enumerate((
                        (weh, xh), (weh, xl), (wel, xh),
                    )):
                        nc.tensor.matmul(
                            out=bank,
                            lhsT=lw[:, g * 128:(g + 1) * 128],
                            rhs=rx[:, base:base + NB],
                            start=(i == 0), stop=(i == 2),
                            skip_group_check=True,
                        )
                if b >= 2:
                    for i, br in enumerate((brh, brl)):
                        nc.tensor.matmul(
                            out=ch[:, G * NB:4 * NB],
                            lhsT=br[0:1, :], rhs=ones[0:1, :],
                            start=(i == 0), stop=False,
                            skip_group_check=True,
                        )
                else:
                    nc.vector.memset(ch[:, G * NB:4 * NB], 0.0)

            def spikes_and_fc(b):
                """After block b's ticks: sigma = Sign(m1 - 1) over its ring
                tile, then fc (6 bf16 matmuls) into bank 3 of C tile b+2."""
                if b < 0 or b >= fcblk:
                    return
                ring3 = rings[b % 2][:].rearrange("p (t c) -> p t c", c=128)
                sg = sgp.tile([128, BLK * G * BC], BF16, tag="sg")
                sg3 = sg[:].rearrange("p (t c) -> p t c", c=G * BC)
                nc.scalar.activation(
                    out=sg3[:], in_=ring3[:, :, 0:G * BC], func=AF.Sign,
                    bias=negthr[:],
                )
                n_mm = 2 * G
                i = 0
                for g in range(G):
                    for lw in (fch, fcl):
                        i += 1
                        nc.tensor.matmul(
                            out=chs[b + 2][0:J, G * NB:4 * NB],
                            lhsT=lw[:, g * J:(g + 1) * J],
                            rhs=sg3[:, :, g * BC:(g + 1) * BC],
                            start=False, stop=(i == n_mm),
                            skip_group_check=True,
                        )

            def hist_copy(b):
                """mem2 of DVE-tick block b = m2 ticks [16b-32, 16b-16)."""
                t0 = b * BLK - SKEW
                if t0 < 0:
                    return
                n = min(BLK, T - t0)
                if n <= 0:
                    return
                ring3 = rings[b % 2][:].rearrange("p (t c) -> p t c", c=128)
                hview = hist[:].rearrange("p (s t) -> p s t", t=T)
                nc.scalar.activation(
                    out=hview[0:J, :, t0:t0 + n].transpose((0, 2, 1)),
                    in_=ring3[0:J, 0:n, G * BC:128], func=AF.Copy,
                )

            # prologue
            load_window(0)
            prep_block(0)
            prep_block(1)

            for t in range(ticks):
                b, lo = divmod(t, BLK)
                if lo == 0:
                    if ((b + 3) * BLK) % WIN == 0:
                        load_window(((b + 3) * BLK) // WIN)
                    prep_block(b + 1)
                    spikes_and_fc(b - 1)
                    hist_copy(b - 1)
                ring = rings[b % 2]
                prev = rings[(b - 1) % 2] if lo == 0 else ring
                plo = (lo - 1) % BLK
                ch4 = chs[b][:].rearrange("p (g t n) -> p g t n", g=4, n=BC)

                nc.vector._custom_dve(
                    lif,
                    out=ring[:, lo * 128:(lo + 1) * 128],
                    in0=prev[:, plo * 128:(plo + 1) * 128],
                    in1=ch4[:, :, lo, :],
                    s0=BETA, s1=THR,
                )

            # epilogue: the last block's mem2 history
            spikes_and_fc(nblk - 1)
            hist_copy(nblk - 1)

            acc_sb = kp.tile([J, BC], F32, tag="acc_sb")
            hview = hist[:].rearrange("p (s t) -> p s t", t=T)
            nc.vector.tensor_reduce(
                out=acc_sb[:], in_=hview[:], axis=AX.X, op=ALU.add,
            )
            nc.sync.dma_start(out_d[:], acc_sb[:])

    nc.compile()
    return nc


def _bf16_split(a):
    import ml_dtypes
    hi = a.astype(ml_dtypes.bfloat16)
    lo = (a - hi.astype(np.float32)).astype(ml_dtypes.bfloat16)
    return hi, lo


def _host_prep(x, conv_w, conv_b, fc_w, fc_b, T):
    """Build per-core input maps (numpy only)."""
    import ml_dtypes
    ticks = T + SKEW
    nblk = -(-ticks // BLK)
    windows = -(-(nblk * BLK) // WIN)
    xt_ticks = windows * WIN

    wexp = np.zeros((KX, F), np.float32)
    for c in range(CH):
        for l in range(LO):
            wexp[l:l + 7, c * LO + l] = conv_w[c, 0, :]
        wexp[L_IN, c * LO:(c + 1) * LO] = conv_b[c]
    weh, wel = _bf16_split(wexp)

    # spike trick: s = (sigma+1)/2 with sigma = sign(m-1) in {-1,0,1}
    # c2 = fc_w @ s + b = (fc_w/2) @ sigma + (b + fc_w.sum/2)
    half = (fc_w * 0.5).astype(np.float32)
    fcwt = np.zeros((128, G * J), np.float32)
    for g in range(G):
        fcwt[:, g * J:(g + 1) * J] = half[:, g * 128:(g + 1) * 128].T
    fch, fcl = _bf16_split(fcwt)
    brow = np.zeros((1, 128), np.float32)
    brow[0, :J] = fc_b + half.sum(axis=1)
    brh, brl = _bf16_split(brow)

    ones = np.ones((1, BLK * BC), ml_dtypes.bfloat16)

    in_maps = []
    B = x.shape[0]
    n_cores = B // BC
    for core in range(n_cores):
        xc = x[core * BC:(core + 1) * BC]          # [BC, T, L]
        xt = np.zeros((KX, xt_ticks, BC), np.float32)
        xt[:L_IN, :T, :] = xc.transpose(2, 1, 0)
        xt[L_IN, :T, :] = 1.0
        xt = xt.reshape(KX, xt_ticks * BC)
        xth, xtl = _bf16_split(xt)
        in_maps.append({
            "xth": xth, "xtl": xtl, "weh": weh, "wel": wel,
            "fch": fch, "fcl": fcl, "brh": brh, "brl": brl, "ones": ones,
        })
    return in_maps


def _install_trace_hook():
    """Wire up the axon NTFF profiling hook (absent from this image)."""
    import types

    if "antenv.axon_hooks" in sys.modules:
        return True
    try:
        if "/root/.axon_site" not in sys.path:
            sys.path.insert(0, "/root/.axon_site")
        from trn_agent_boot.trn_boot import _ntff_profile_via_ctypes

        hook = _ntff_profile_via_ctypes("/opt/axon/libaxon_pjrt.so")
        if hook is None:
            return False
        mod = types.ModuleType("antenv.axon_hooks")
        mod.get_axon_ntff_profile_hook = lambda: hook
        sys.modules["antenv.axon_hooks"] = mod
        import concourse.bass_utils as bu

        bu.upload_artifacts = lambda tmpdir: str(tmpdir)
        return True
    except Exception as e:  # profiling is optional
        print(f"trace hook install failed: {e}", file=sys.stderr)
        return False


def run_cores(x, conv_w, conv_b, fc_w, fc_b, T=None):
    """Run the Bass kernel on len(batch)/32 cores; returns [B, 35] output."""
    global LAST_RESULTS
    T = T if T is not None else x.shape[1]
    trace = TRACE and _install_trace_hook()
    nc = _build_nc(T)
    in_maps = _host_prep(x, conv_w, conv_b, fc_w, fc_b, T)
    res = run_bass_kernel_spmd(
        nc, in_maps, core_ids=list(range(len(in_maps))), trace=trace,
    )
    LAST_RESULTS = res
    outs = []
    for i in range(len(in_maps)):
        acc_v = res.results[i]["out"]              # [J, BC] = sum_t mem2
        outs.append((acc_v.T / np.float32(T)).astype(np.float32))
    return np.concatenate(outs, axis=0)


def kernel(x, conv_w, conv_b, fc_w, fc_b):
    return run_cores(
        np.asarray(x, np.float32), np.asarray(conv_w, np.float32),
        np.asarray(conv_b, np.float32), np.asarray(fc_w, np.float32),
        np.asarray(fc_b, np.float32),
    )


# revision 19
# speedup vs baseline: 3.4239x; 1.0306x over previous
"""Trainium2 Bass kernel for nn_BasicClassifier (spiking conv classifier).

Sharding: pure data parallelism — batch 256 is split 32 samples per core
across 8 NeuronCores; params are replicated (tiny).

Per-core design. The T=1000 LIF scan is sequential, so per-step cost on the
pacing engine decides everything. State lives in a ring of [128, 128] fp32
SBUF slices (2 block-sized ring tiles, 16 slices each):
  cols 0:96   layer-1 membrane, feature f = g*128+p at (p, g*32+b), g in 0..2
  cols 96:128 layer-2 membrane [35 units x 32 samples], lagged SKEW=32 ticks
Per tick the whole LIF update (leak + input + reset-by-subtraction) is ONE
fused custom-DVE op:  m' = (m*0.9 + c) - (m > 1)   (the spike derives from
state, so the DVE chain never waits on other engines mid-block). Its input
drive c is read straight out of a 4-bank PSUM block tile via a strided
[128, 4, 32] access pattern — no assembly copies.

Per 16-tick block (all off the tick-critical path, bf16 hi/lo splits keep
matmuls single-pass while preserving ~2^-16 relative precision):
  - C tile = PSUM [128, 4*512]: banks 0-2 = conv1d hoisted into [31 -> 384]
    GEMMs (3 products: xh@Wh + xl@Wh + xh@Wl; ones row folds conv_b),
    bank 3 = fc bias prefill (bh+bl) + fc results of block b-2.
  - spikes: ONE ACT Sign op per block over the ring -> sigma in {-1,0,1} bf16;
    (sigma+1)/2 is folded into halved fc weights + adjusted fc bias.
  - fc 384->35: 6 bf16 matmuls of N=512 (hi/lo x 3 K-chunks) accumulating
    onto the bias in bank 3 of the C tile two blocks ahead.
  - mem2 history: ACT copies ring cols 96:128 to an fp16 [35, 32*T] buffer;
    one final DVE tensor_reduce produces sum_t mem2.
"""

import os
import sys

for _p in ("/opt/trn_rl_repo", "/opt/pypackages"):
    if _p not in sys.path:
        sys.path.insert(0, _p)

import numpy as np

import concourse.bacc as bacc
import concourse.mybir as mybir
import concourse.tile as tile
import concourse.dve_ops as dve_ops
from concourse.dve_spec import Spec, Src0, Src1, C0, C1, lower
from concourse.dve_uop import DveOpSpec
from concourse.bass_utils import run_bass_kernel_spmd

F32 = mybir.dt.float32
F16 = mybir.dt.float16
BF16 = mybir.dt.bfloat16
ALU = mybir.AluOpType
AF = mybir.ActivationFunctionType
AX = mybir.AxisListType

N_CORES = 8
B_FULL, T_FULL, L_IN = 256, 1000, 30
BC = B_FULL // N_CORES      # 32 samples per core
CH, LO = 16, 24
F = CH * LO                 # 384 features
G = 3                       # feature groups of 128
J = 35                      # fc outputs
KX = L_IN + 1               # conv contraction rows (30 taps + ones row)
BLK = 16                    # ticks per block (N = 16*32 = 512 = 1 PSUM bank)
SKEW = 2 * BLK              # layer-2 lag: c2_t consumed at DVE tick t+SKEW
WIN = 160                   # ticks per x-window DMA (multiple of BLK)
BETA, THR = 0.9, 1.0

TRACE = bool(int(os.environ.get("KERNEL_TRACE", "0")))
LAST_RESULTS = None

_LIF_OP = None


def _get_lif_op():
    """Register the fused LIF-step op in the custom-DVE table (idempotent)."""
    global _LIF_OP
    if _LIF_OP is not None:
        return _LIF_OP
    name = "LIF_STEP_ANT59"
    for op in dve_ops.OPS:
        if op.name == name:
            _LIF_OP = op
            return op
    spec = Spec(
        body=(Src0 * C0 + Src1) - (Src0 > C1),
        reference=lambda in0, in1, s0, s1, imm2: (
            (in0.astype(np.float32) * np.float32(s0)
             + in1.reshape(in0.shape))
            - (in0 > s1).astype(np.float32)
        ).astype(np.float32),
    )
    row = dve_ops._CUSTOM_DVE_ROW_BASE + len(dve_ops.OPS)
    assert row < 0x20
    dve_ops._SUB_OPCODE_FOR_NAME[name] = row
    shas = {}
    for ver in ("v3", "v4"):
        tmp = DveOpSpec(name=name, opcode=row, uops=lower(spec, ver=ver), rd1_en=True)
        shas[ver] = tmp.sha(ver)
    op = dve_ops.DveOp(name, spec, subdim=False, uops_sha=shas)
    dve_ops.OPS.append(op)
    dve_ops.CUSTOM_DVE_SPECS[name] = spec
    _LIF_OP = op
    return op


def _build_nc(T):
    """Build the per-core Bass program (SPMD: same program on every core)."""
    lif = _get_lif_op()
    ticks = T + SKEW                       # DVE ticks 0..T+SKEW-1
    nblk = -(-ticks // BLK)
    pad_ticks = nblk * BLK
    windows = -(-pad_ticks // WIN)
    xt_cols = windows * WIN * BC
    fcblk = -(-T // BLK)                   # blocks that need spikes/fc
    NB = BLK * BC                          # 512: one PSUM bank of f32

    nc = bacc.Bacc("TRN2", target_bir_lowering=False)

    KS = 3 * KX                            # stacked conv K: [xh; xl; xh]
    xts_d = nc.dram_tensor("xts", [KS, xt_cols], BF16, kind="ExternalInput")
    wes_d = nc.dram_tensor("wes", [KS, F], BF16, kind="ExternalInput")
    fch_d = nc.dram_tensor("fch", [128, G * J], BF16, kind="ExternalInput")
    fcl_d = nc.dram_tensor("fcl", [128, G * J], BF16, kind="ExternalInput")
    brs_d = nc.dram_tensor("brs", [2, 128], BF16, kind="ExternalInput")
    ones_d = nc.dram_tensor("ones", [2, NB], BF16, kind="ExternalInput")
    out_d = nc.dram_tensor("out", [J, BC], F32, kind="ExternalOutput")

    with tile.TileContext(nc) as tc:
        with (
            tc.tile_pool(name="konst", bufs=1) as kp,
            tc.tile_pool(name="ring", bufs=1) as rp,
            tc.tile_pool(name="hist", bufs=1) as hp,
            tc.tile_pool(name="sig", bufs=2) as sgp,
            tc.tile_pool(name="xwin", bufs=2) as xp,
            tc.tile_pool(name="cpsum", bufs=2, space="PSUM") as cp,
        ):
            # constants -> SBUF
            wes = kp.tile([KS, F], BF16, tag="wes")
            fch = kp.tile([128, G * J], BF16, tag="fch")
            fcl = kp.tile([128, G * J], BF16, tag="fcl")
            brs = kp.tile([2, 128], BF16, tag="brs")
            ones = kp.tile([2, NB], BF16, tag="ones")
            negthr = kp.tile([128, 1], F32, tag="negthr")
            nc.vector.memset(negthr[:], -THR)
            for sb, dr in ((wes, wes_d), (fch, fch_d), (fcl, fcl_d),
                           (brs, brs_d), (ones, ones_d)):
                nc.sync.dma_start(sb[:], dr[:])

            # state ring: 2 block-sized tiles of 16 slices each
            ringA = rp.tile([128, BLK * 128], F32, tag="ringA")
            ringB = rp.tile([128, BLK * 128], F32, tag="ringB")
            nc.vector.memset(ringA[:], 0.0)
            nc.vector.memset(ringB[:], 0.0)
            rings = (ringA, ringB)

            # mem2 history (fp16), col = sample*T + t
            hist = hp.tile([J, BC * T], F16, tag="hist")

            xts = {}      # window idx -> (hi, lo) xt sbuf tiles
            chs = {}      # block idx -> PSUM C tile [128, 4*512]

            def load_window(w):
                if w >= windows or w in xts:
                    return
                ts = xp.tile([KS, WIN * BC], BF16, tag="xws")
                nc.sync.dma_start(ts[:], xts_d[:, w * WIN * BC:(w + 1) * WIN * BC])
                xts[w] = ts

            def prep_block(b):
                """Allocate block b's PSUM C tile; fill conv banks 0-2 and
                bias-prefill bank 3 (fc of block b-2 lands there later)."""
                if b >= nblk or b in chs:
                    return
                w = (b * BLK) // WIN
                base = (b * BLK - w * WIN) * BC
                xw = xts[w]
                ch = cp.tile([128, 4 * NB], F32, tag="ch")
                chs[b] = ch
                for g in range(G):
                    nc.tensor.matmul(
                        out=ch[:, g * NB:(g + 1) * NB],
                        lhsT=wes[:, g * 128:(g + 1) * 128],
                        rhs=xw[:, base:base + NB],
                        start=True, stop=True,
                    )
                if b >= 2:
                    nc.tensor.matmul(
                        out=ch[:, G * NB:4 * NB],
                        lhsT=brs[:, :], rhs=ones[:, :],
                        start=True, stop=False,
                        skip_group_check=True,
                    )
                else:
                    nc.vector.memset(ch[:, G * NB:4 * NB], 0.0)

            def spikes_and_fc(b):
                """After block b's ticks: sigma = Sign(m1 - 1) over its ring
                tile, then fc (6 bf16 matmuls) into bank 3 of C tile b+2."""
                if b < 0 or b >= fcblk:
                    return
                ring3 = rings[b % 2][:].rearrange("p (t c) -> p t c", c=128)
                sg = sgp.tile([128, BLK * G * BC], BF16, tag="sg")
                sg3 = sg[:].rearrange("p (t c) -> p t c", c=G * BC)
                nc.scalar.activation(
                    out=sg3[:], in_=ring3[:, :, 0:G * BC], func=AF.Sign,
                    bias=negthr[:],
                )
                n_mm = 2 * G
                i = 0
                for g in range(G):
                    for lw in (fch, fcl):
                        i += 1
                        nc.tensor.matmul(
                            out=chs[b + 2][0:J, G * NB:4 * NB],
                            lhsT=lw[:, g * J:(g + 1) * J],
                            rhs=sg3[:, :, g * BC:(g + 1) * BC],
                            start=False, stop=(i == n_mm),
                            skip_group_check=True,
                        )

            def hist_copy(b):
                """mem2 of DVE-tick block b = m2 ticks [16b-32, 16b-16)."""
                t0 = b * BLK - SKEW
                if t0 < 0:
                    return
                n = min(BLK, T - t0)
                if n <= 0:
                    return
                ring3 = rings[b % 2][:].rearrange("p (t c) -> p t c", c=128)
                hview = hist[:].rearrange("p (s t) -> p s t", t=T)
                nc.scalar.activation(
                    out=hview[0:J, :, t0:t0 + n].transpose((0, 2, 1)),
                    in_=ring3[0:J, 0:n, G * BC:128], func=AF.Copy,
                )

            acc_parts = []
            red_done = 0

            def partial_reduce(t_hi):
                """Reduce hist[:, :, red_done:t_hi] into a partial acc tile."""
                nonlocal red_done
                if t_hi <= red_done:
                    return
                part = kp.tile([J, BC], F32, tag=f"accp{len(acc_parts)}")
                acc_parts.append(part)
                hv = hist[:].rearrange("p (s t) -> p s t", t=T)
                nc.vector.tensor_reduce(
                    out=part[:], in_=hv[:, :, red_done:t_hi], axis=AX.X,
                    op=ALU.add,
                )
                red_done = t_hi

            # prologue
            load_window(0)
            prep_block(0)
            prep_block(1)

            for t in range(ticks):
                b, lo = divmod(t, BLK)
                if lo == 0:
                    if ((b + 3) * BLK) % WIN == 0:
                        load_window(((b + 3) * BLK) // WIN)
                    prep_block(b + 1)
                    spikes_and_fc(b - 1)
                    hist_copy(b - 1)
                    # hist of ticks < (b-1)*BLK-SKEW is final: fold it in
                    avail = (b - 1) * BLK - SKEW
                    if avail - red_done >= 256 and avail < T:
                        partial_reduce(avail)
                ring = rings[b % 2]
                prev = rings[(b - 1) % 2] if lo == 0 else ring
                plo = (lo - 1) % BLK
                ch4 = chs[b][:].rearrange("p (g t n) -> p g t n", g=4, n=BC)

                nc.vector._custom_dve(
                    lif,
                    out=ring[:, lo * 128:(lo + 1) * 128],
                    in0=prev[:, plo * 128:(plo + 1) * 128],
                    in1=ch4[:, :, lo, :],
                    s0=BETA, s1=THR,
                )

            # epilogue: the last block's mem2 history
            spikes_and_fc(nblk - 1)
            hist_copy(nblk - 1)

            partial_reduce(T)
            acc_sb = kp.tile([J, BC], F32, tag="acc_sb")
            nc.vector.tensor_copy(acc_sb[:], acc_parts[0][:])
            for part in acc_parts[1:]:
                nc.vector.tensor_tensor(
                    out=acc_sb[:], in0=acc_sb[:], in1=part[:], op=ALU.add,
                )
            nc.sync.dma_start(out_d[:], acc_sb[:])

    nc.compile()
    return nc


def _bf16_split(a):
    import ml_dtypes
    hi = a.astype(ml_dtypes.bfloat16)
    lo = (a - hi.astype(np.float32)).astype(ml_dtypes.bfloat16)
    return hi, lo


def _host_prep(x, conv_w, conv_b, fc_w, fc_b, T):
    """Build per-core input maps (numpy only)."""
    import ml_dtypes
    ticks = T + SKEW
    nblk = -(-ticks // BLK)
    windows = -(-(nblk * BLK) // WIN)
    xt_ticks = windows * WIN

    wexp = np.zeros((KX, F), np.float32)
    for c in range(CH):
        for l in range(LO):
            wexp[l:l + 7, c * LO + l] = conv_w[c, 0, :]
        wexp[L_IN, c * LO:(c + 1) * LO] = conv_b[c]
    weh, wel = _bf16_split(wexp)
    wes = np.concatenate([weh, weh, wel], axis=0)  # K-stacked [93, F]

    # spike trick: s = (sigma+1)/2 with sigma = sign(m-1) in {-1,0,1}
    # c2 = fc_w @ s + b = (fc_w/2) @ sigma + (b + fc_w.sum/2)
    half = (fc_w * 0.5).astype(np.float32)
    fcwt = np.zeros((128, G * J), np.float32)
    for g in range(G):
        fcwt[:, g * J:(g + 1) * J] = half[:, g * 128:(g + 1) * 128].T
    fch, fcl = _bf16_split(fcwt)
    brow = np.zeros((1, 128), np.float32)
    brow[0, :J] = fc_b + half.sum(axis=1)
    brh, brl = _bf16_split(brow)
    brs = np.concatenate([brh, brl], axis=0)       # [2, 128]

    ones = np.ones((2, BLK * BC), ml_dtypes.bfloat16)

    in_maps = []
    B = x.shape[0]
    n_cores = B // BC
    for core in range(n_cores):
        xc = x[core * BC:(core + 1) * BC]          # [BC, T, L]
        xt = np.zeros((KX, xt_ticks, BC), np.float32)
        xt[:L_IN, :T, :] = xc.transpose(2, 1, 0)
        xt[L_IN, :T, :] = 1.0
        xt = xt.reshape(KX, xt_ticks * BC)
        xth, xtl = _bf16_split(xt)
        xstk = np.concatenate([xth, xtl, xth], axis=0)  # [93, cols]
        in_maps.append({
            "xts": xstk, "wes": wes, "fch": fch, "fcl": fcl,
            "brs": brs, "ones": ones,
        })
    return in_maps


def _install_trace_hook():
    """Wire up the axon NTFF profiling hook (absent from this image)."""
    import types

    if "antenv.axon_hooks" in sys.modules:
        return True
    try:
        if "/root/.axon_site" not in sys.path:
            sys.path.insert(0, "/root/.axon_site")
        from trn_agent_boot.trn_boot import _ntff_profile_via_ctypes

        hook = _ntff_profile_via_ctypes("/opt/axon/libaxon_pjrt.so")
        if hook is None:
            return False
        mod = types.ModuleType("antenv.axon_hooks")
        mod.get_axon_ntff_profile_hook = lambda: hook
        sys.modules["antenv.axon_hooks"] = mod
        import concourse.bass_utils as bu

        bu.upload_artifacts = lambda tmpdir: str(tmpdir)
        return True
    except Exception as e:  # profiling is optional
        print(f"trace hook install failed: {e}", file=sys.stderr)
        return False


def run_cores(x, conv_w, conv_b, fc_w, fc_b, T=None):
    """Run the Bass kernel on len(batch)/32 cores; returns [B, 35] output."""
    global LAST_RESULTS
    T = T if T is not None else x.shape[1]
    trace = TRACE and _install_trace_hook()
    nc = _build_nc(T)
    in_maps = _host_prep(x, conv_w, conv_b, fc_w, fc_b, T)
    res = run_bass_kernel_spmd(
        nc, in_maps, core_ids=list(range(len(in_maps))), trace=trace,
    )
    LAST_RESULTS = res
    outs = []
    for i in range(len(in_maps)):
        acc_v = res.results[i]["out"]              # [J, BC] = sum_t mem2
        outs.append((acc_v.T / np.float32(T)).astype(np.float32))
    return np.concatenate(outs, axis=0)


def kernel(x, conv_w, conv_b, fc_w, fc_b):
    return run_cores(
        np.asarray(x, np.float32), np.asarray(conv_w, np.float32),
        np.asarray(conv_b, np.float32), np.asarray(fc_w, np.float32),
        np.asarray(fc_b, np.float32),
    )


# revision 20
# speedup vs baseline: 4.2783x; 1.2496x over previous
"""Trainium2 Bass kernel for nn_BasicClassifier (spiking conv classifier).

Sharding: pure data parallelism — batch 256 is split 32 samples per core
across 8 NeuronCores; params are replicated (tiny).

Per-core design. The T=1000 LIF scan is sequential, so per-step cost on the
pacing engine decides everything. State lives in a ring of [128, 128] fp32
SBUF slices (2 block-sized ring tiles, 16 slices each):
  cols 0:96   layer-1 membrane, feature f = g*128+p at (p, g*32+b), g in 0..2
  cols 96:128 layer-2 membrane [35 units x 32 samples], lagged SKEW=32 ticks
Per tick the whole LIF update (leak + input + reset-by-subtraction) is ONE
fused custom-DVE op:  m' = (m*0.9 + c) - (m > 1)   (the spike derives from
state, so the DVE chain never waits on other engines mid-block). Its input
drive c is read straight out of a 4-bank PSUM block tile via a strided
[128, 4, 32] access pattern — no assembly copies.

Per 16-tick block (all off the tick-critical path, bf16 hi/lo splits keep
matmuls single-pass while preserving ~2^-16 relative precision):
  - C tile = PSUM [128, 4*512]: banks 0-2 = conv1d hoisted into [31 -> 384]
    GEMMs (3 products: xh@Wh + xl@Wh + xh@Wl; ones row folds conv_b),
    bank 3 = fc bias prefill (bh+bl) + fc results of block b-2.
  - spikes: ONE ACT Sign op per block over the ring -> sigma in {-1,0,1} bf16;
    (sigma+1)/2 is folded into halved fc weights + adjusted fc bias.
  - fc 384->35: 6 bf16 matmuls of N=512 (hi/lo x 3 K-chunks) accumulating
    onto the bias in bank 3 of the C tile two blocks ahead.
  - mem2 history: ACT copies ring cols 96:128 to an fp16 [35, 32*T] buffer;
    one final DVE tensor_reduce produces sum_t mem2.
"""

import os
import sys

for _p in ("/opt/trn_rl_repo", "/opt/pypackages"):
    if _p not in sys.path:
        sys.path.insert(0, _p)

import numpy as np

import concourse.bacc as bacc
import concourse.mybir as mybir
import concourse.tile as tile
import concourse.dve_ops as dve_ops
from concourse.dve_spec import Spec, Src0, Src1, C0, C1, lower
from concourse.dve_uop import DveOpSpec
from concourse.bass_utils import run_bass_kernel_spmd

F32 = mybir.dt.float32
F16 = mybir.dt.float16
BF16 = mybir.dt.bfloat16
ALU = mybir.AluOpType
AF = mybir.ActivationFunctionType
AX = mybir.AxisListType

N_CORES = 8
B_FULL, T_FULL, L_IN = 256, 1000, 30
BC = B_FULL // N_CORES      # 32 samples per core
CH, LO = 16, 24
F = CH * LO                 # 384 features
G = 3                       # feature groups of 128
J = 35                      # fc outputs
KX = L_IN + 1               # conv contraction rows (30 taps + ones row)
BLK = 16                    # ticks per block (N = 16*32 = 512 = 1 PSUM bank)
SKEW = 2 * BLK              # layer-2 lag: c2_t consumed at DVE tick t+SKEW
WIN = 160                   # ticks per x-window DMA (multiple of BLK)
BETA, THR = 0.9, 1.0

TRACE = bool(int(os.environ.get("KERNEL_TRACE", "0")))
LAST_RESULTS = None

_LIF_OP = None


def _get_lif_op():
    """Register the fused LIF-step op in the custom-DVE table (idempotent)."""
    global _LIF_OP
    if _LIF_OP is not None:
        return _LIF_OP
    name = "LIF_STEP_ANT59"
    for op in dve_ops.OPS:
        if op.name == name:
            _LIF_OP = op
            return op
    spec = Spec(
        body=(Src0 * C0 + Src1) - (Src0 > C1),
        reference=lambda in0, in1, s0, s1, imm2: (
            (in0.astype(np.float32) * np.float32(s0)
             + in1.reshape(in0.shape))
            - (in0 > s1).astype(np.float32)
        ).astype(np.float32),
    )
    row = dve_ops._CUSTOM_DVE_ROW_BASE + len(dve_ops.OPS)
    assert row < 0x20
    dve_ops._SUB_OPCODE_FOR_NAME[name] = row
    shas = {}
    for ver in ("v3", "v4"):
        tmp = DveOpSpec(name=name, opcode=row, uops=lower(spec, ver=ver), rd1_en=True)
        shas[ver] = tmp.sha(ver)
    op = dve_ops.DveOp(name, spec, subdim=False, uops_sha=shas)
    dve_ops.OPS.append(op)
    dve_ops.CUSTOM_DVE_SPECS[name] = spec
    _LIF_OP = op
    return op


def _build_nc(T):
    """Build the per-core Bass program (SPMD: same program on every core)."""
    lif = _get_lif_op()
    ticks = T + SKEW                       # DVE ticks 0..T+SKEW-1
    nblk = -(-ticks // BLK)
    pad_ticks = nblk * BLK
    windows = -(-pad_ticks // WIN)
    xt_cols = windows * WIN * BC
    fcblk = -(-T // BLK)                   # blocks that need spikes/fc
    NB = BLK * BC                          # 512: one PSUM bank of f32

    nc = bacc.Bacc("TRN2", target_bir_lowering=False)

    KS = 3 * KX                            # stacked conv K: [xh; xl; xh]
    xts_d = nc.dram_tensor("xts", [KS, xt_cols], BF16, kind="ExternalInput")
    wes_d = nc.dram_tensor("wes", [KS, F], BF16, kind="ExternalInput")
    fch_d = nc.dram_tensor("fch", [128, G * J], BF16, kind="ExternalInput")
    fcl_d = nc.dram_tensor("fcl", [128, G * J], BF16, kind="ExternalInput")
    brs_d = nc.dram_tensor("brs", [2, 128], BF16, kind="ExternalInput")
    ones_d = nc.dram_tensor("ones", [2, NB], BF16, kind="ExternalInput")
    hist_d = nc.dram_tensor("hist", [J, BC * T], F16, kind="ExternalOutput")

    with tile.TileContext(nc) as tc:
        with (
            tc.tile_pool(name="konst", bufs=1) as kp,
            tc.tile_pool(name="ring", bufs=1) as rp,
            tc.tile_pool(name="hstage", bufs=2) as hp,
            tc.tile_pool(name="sig", bufs=2) as sgp,
            tc.tile_pool(name="xwin", bufs=2) as xp,
            tc.tile_pool(name="cpsum", bufs=2, space="PSUM") as cp,
        ):
            # constants -> SBUF
            wes = kp.tile([KS, F], BF16, tag="wes")
            fch = kp.tile([128, G * J], BF16, tag="fch")
            fcl = kp.tile([128, G * J], BF16, tag="fcl")
            brs = kp.tile([2, 128], BF16, tag="brs")
            ones = kp.tile([2, NB], BF16, tag="ones")
            negthr = kp.tile([128, 1], F32, tag="negthr")
            nc.vector.memset(negthr[:], -THR)
            for sb, dr in ((wes, wes_d), (fch, fch_d), (fcl, fcl_d),
                           (brs, brs_d), (ones, ones_d)):
                nc.sync.dma_start(sb[:], dr[:])

            # state ring: 2 block-sized tiles of 16 slices each
            ringA = rp.tile([128, BLK * 128], F32, tag="ringA")
            ringB = rp.tile([128, BLK * 128], F32, tag="ringB")
            nc.vector.memset(ringA[:], 0.0)
            nc.vector.memset(ringB[:], 0.0)
            rings = (ringA, ringB)


            xts = {}      # window idx -> (hi, lo) xt sbuf tiles
            chs = {}      # block idx -> PSUM C tile [128, 4*512]

            def load_window(w):
                if w >= windows or w in xts:
                    return
                ts = xp.tile([KS, WIN * BC], BF16, tag="xws")
                nc.sync.dma_start(ts[:], xts_d[:, w * WIN * BC:(w + 1) * WIN * BC])
                xts[w] = ts

            def prep_block(b):
                """Allocate block b's PSUM C tile; fill conv banks 0-2 and
                bias-prefill bank 3 (fc of block b-2 lands there later)."""
                if b >= nblk or b in chs:
                    return
                w = (b * BLK) // WIN
                base = (b * BLK - w * WIN) * BC
                xw = xts[w]
                ch = cp.tile([128, 4 * NB], F32, tag="ch")
                chs[b] = ch
                for g in range(G):
                    nc.tensor.matmul(
                        out=ch[:, g * NB:(g + 1) * NB],
                        lhsT=wes[:, g * 128:(g + 1) * 128],
                        rhs=xw[:, base:base + NB],
                        start=True, stop=True,
                    )
                if b >= 2:
                    nc.tensor.matmul(
                        out=ch[:, G * NB:4 * NB],
                        lhsT=brs[:, :], rhs=ones[:, :],
                        start=True, stop=False,
                        skip_group_check=True,
                    )
                else:
                    nc.vector.memset(ch[:, G * NB:4 * NB], 0.0)

            def spikes_and_fc(b):
                """After block b's ticks: sigma = Sign(m1 - 1) over its ring
                tile, then fc (6 bf16 matmuls) into bank 3 of C tile b+2."""
                if b < 0 or b >= fcblk:
                    return
                ring3 = rings[b % 2][:].rearrange("p (t c) -> p t c", c=128)
                sg = sgp.tile([128, G * NB], BF16, tag="sg")
                for g in range(G):
                    nc.scalar.activation(
                        out=sg[:, g * NB:(g + 1) * NB],
                        in_=ring3[:, :, g * BC:(g + 1) * BC], func=AF.Sign,
                        bias=negthr[:],
                    )
                n_mm = 2 * G
                i = 0
                for g in range(G):
                    for lw in (fch, fcl):
                        i += 1
                        nc.tensor.matmul(
                            out=chs[b + 2][0:J, G * NB:4 * NB],
                            lhsT=lw[:, g * J:(g + 1) * J],
                            rhs=sg[:, g * NB:(g + 1) * NB],
                            start=False, stop=(i == n_mm),
                            skip_group_check=True,
                        )

            def hist_copy(b):
                """mem2 of DVE-tick block b = m2 ticks [16b-32, 16b-16):
                stage to fp16 then DMA out (host computes the mean)."""
                t0 = b * BLK - SKEW
                if t0 < 0:
                    return
                n = min(BLK, T - t0)
                if n <= 0:
                    return
                ring3 = rings[b % 2][:].rearrange("p (t c) -> p t c", c=128)
                stage = hp.tile([J, BLK * BC], F16, tag="hstage")
                nc.scalar.activation(
                    out=stage[0:J, 0:n * BC],
                    in_=ring3[0:J, 0:n, G * BC:128], func=AF.Copy,
                )
                nc.sync.dma_start(
                    hist_d[:, t0 * BC:(t0 + n) * BC], stage[0:J, 0:n * BC],
                )

            # prologue
            load_window(0)
            prep_block(0)
            prep_block(1)

            for t in range(ticks):
                b, lo = divmod(t, BLK)
                if lo == 0:
                    if ((b + 3) * BLK) % WIN == 0:
                        load_window(((b + 3) * BLK) // WIN)
                    prep_block(b + 1)
                    spikes_and_fc(b - 1)
                    hist_copy(b - 1)
                ring = rings[b % 2]
                prev = rings[(b - 1) % 2] if lo == 0 else ring
                plo = (lo - 1) % BLK
                ch4 = chs[b][:].rearrange("p (g t n) -> p g t n", g=4, n=BC)

                nc.vector._custom_dve(
                    lif,
                    out=ring[:, lo * 128:(lo + 1) * 128],
                    in0=prev[:, plo * 128:(plo + 1) * 128],
                    in1=ch4[:, :, lo, :],
                    s0=BETA, s1=THR,
                )

            # epilogue: the last block's mem2 history
            spikes_and_fc(nblk - 1)
            hist_copy(nblk - 1)



    nc.compile()
    return nc


def _bf16_split(a):
    import ml_dtypes
    hi = a.astype(ml_dtypes.bfloat16)
    lo = (a - hi.astype(np.float32)).astype(ml_dtypes.bfloat16)
    return hi, lo


def _host_prep(x, conv_w, conv_b, fc_w, fc_b, T):
    """Build per-core input maps (numpy only)."""
    import ml_dtypes
    ticks = T + SKEW
    nblk = -(-ticks // BLK)
    windows = -(-(nblk * BLK) // WIN)
    xt_ticks = windows * WIN

    wexp = np.zeros((KX, F), np.float32)
    for c in range(CH):
        for l in range(LO):
            wexp[l:l + 7, c * LO + l] = conv_w[c, 0, :]
        wexp[L_IN, c * LO:(c + 1) * LO] = conv_b[c]
    weh, wel = _bf16_split(wexp)
    wes = np.concatenate([weh, weh, wel], axis=0)  # K-stacked [93, F]

    # spike trick: s = (sigma+1)/2 with sigma = sign(m-1) in {-1,0,1}
    # c2 = fc_w @ s + b = (fc_w/2) @ sigma + (b + fc_w.sum/2)
    half = (fc_w * 0.5).astype(np.float32)
    fcwt = np.zeros((128, G * J), np.float32)
    for g in range(G):
        fcwt[:, g * J:(g + 1) * J] = half[:, g * 128:(g + 1) * 128].T
    fch, fcl = _bf16_split(fcwt)
    brow = np.zeros((1, 128), np.float32)
    brow[0, :J] = fc_b + half.sum(axis=1)
    brh, brl = _bf16_split(brow)
    brs = np.concatenate([brh, brl], axis=0)       # [2, 128]

    ones = np.ones((2, BLK * BC), ml_dtypes.bfloat16)

    in_maps = []
    B = x.shape[0]
    n_cores = B // BC
    for core in range(n_cores):
        xc = x[core * BC:(core + 1) * BC]          # [BC, T, L]
        xt = np.zeros((KX, xt_ticks, BC), np.float32)
        xt[:L_IN, :T, :] = xc.transpose(2, 1, 0)
        xt[L_IN, :T, :] = 1.0
        xt = xt.reshape(KX, xt_ticks * BC)
        xth, xtl = _bf16_split(xt)
        xstk = np.concatenate([xth, xtl, xth], axis=0)  # [93, cols]
        in_maps.append({
            "xts": xstk, "wes": wes, "fch": fch, "fcl": fcl,
            "brs": brs, "ones": ones,
        })
    return in_maps


def _install_trace_hook():
    """Wire up the axon NTFF profiling hook (absent from this image)."""
    import types

    if "antenv.axon_hooks" in sys.modules:
        return True
    try:
        if "/root/.axon_site" not in sys.path:
            sys.path.insert(0, "/root/.axon_site")
        from trn_agent_boot.trn_boot import _ntff_profile_via_ctypes

        hook = _ntff_profile_via_ctypes("/opt/axon/libaxon_pjrt.so")
        if hook is None:
            return False
        mod = types.ModuleType("antenv.axon_hooks")
        mod.get_axon_ntff_profile_hook = lambda: hook
        sys.modules["antenv.axon_hooks"] = mod
        import concourse.bass_utils as bu

        bu.upload_artifacts = lambda tmpdir: str(tmpdir)
        return True
    except Exception as e:  # profiling is optional
        print(f"trace hook install failed: {e}", file=sys.stderr)
        return False


def run_cores(x, conv_w, conv_b, fc_w, fc_b, T=None):
    """Run the Bass kernel on len(batch)/32 cores; returns [B, 35] output."""
    global LAST_RESULTS
    T = T if T is not None else x.shape[1]
    trace = TRACE and _install_trace_hook()
    nc = _build_nc(T)
    in_maps = _host_prep(x, conv_w, conv_b, fc_w, fc_b, T)
    res = run_bass_kernel_spmd(
        nc, in_maps, core_ids=list(range(len(in_maps))), trace=trace,
    )
    LAST_RESULTS = res
    outs = []
    for i in range(len(in_maps)):
        hv = np.asarray(res.results[i]["hist"], dtype=np.float32)
        m2 = hv.reshape(J, T, BC)                  # [J, t, sample]
        outs.append((m2.sum(axis=1) / np.float32(T)).T.astype(np.float32))
    return np.concatenate(outs, axis=0)


def kernel(x, conv_w, conv_b, fc_w, fc_b):
    return run_cores(
        np.asarray(x, np.float32), np.asarray(conv_w, np.float32),
        np.asarray(conv_b, np.float32), np.asarray(fc_w, np.float32),
        np.asarray(fc_b, np.float32),
    )
